# revision 1
# baseline (speedup 1.0000x reference)
"""Trainium2 Bass kernel for nn_MultiHeadAttention_21251498181338.

Music-Transformer-style MHA with relative position embeddings (Huang et al.
skew trick), B=2, L=2048, D=1024, H=16, causal mask.

Sharding: 8 cores = 2 batches x 4-head groups (tensor parallel per head).
Each core computes q/k/v projections for its 4 heads, causal attention with
relative position logits, and a partial output projection (Wo row-split).
Partials are summed on the host during unshard.

Device-side structure (per core):
  - Projections produce qh^T/kh^T in [head-depth on partitions] layout and
    vh in [keys on partitions] layout, so no transposes are needed anywhere
    except for the attention probabilities themselves.
  - P = exp(QK^T/8) * exp(Srel/8): the additive logit split is computed
    multiplicatively so the relative-position term can be skew-aligned
    independently of QK^T.
  - The skew is a single SBUF->SBUF DMA per (head, q-tile) using a flat
    access pattern with partition step (row_len - 1): row i is read with a
    column offset of -i, which is exactly the Huang et al. pad/reshape
    trick. Columns beyond the valid relative-index range are zeroed, which
    also implements the causal mask for free (P = Pqk * 0 = 0 there).
  - PV uses TensorE transposes of P tiles; a parallel all-ones-stationary
    matmul produces the softmax denominators, replicated across psum
    partitions so the normalization is a single aligned vector multiply.
  - The two heads of each pair interleave their K=64 matmuls (different PE
    row-groups run concurrently) and share [128, P] psum tiles for PV and
    denominators via tile_position column halves, so both heads normalize
    in one op and land directly in the packed outT layout.
  - The attention output appears transposed [depth, queries], which is
    exactly the stationary-operand layout the output projection needs.
"""

import os
import sys

sys.path.insert(0, "/opt/trn_rl_repo")

import numpy as np
import ml_dtypes

import concourse.bass as bass
import concourse.mybir as mybir
import concourse.tile as tile
from concourse import bacc
from concourse.bass_utils import run_bass_kernel_spmd
from concourse.masks import make_identity

BF16 = mybir.dt.bfloat16
F32 = mybir.dt.float32
NPBF16 = ml_dtypes.bfloat16

B, L, DM, H, D = 2, 2048, 1024, 16, 64
HG = 4            # heads per core (head group)
NCORES = 8
P = 128
KT = DM // P      # 8 contraction tiles for projections
NIT = L // P      # 16 query tiles
SCALE = 1.0 / np.sqrt(D)  # 0.125

LAST_EXEC_NS = None

_PROG = None


def _ncj(it):
    # number of 512-wide key chunks for query tile `it` (causal)
    return it // 4 + 1


def build_program():
    nc = bacc.Bacc(
        "TRN2",
        target_bir_lowering=False,
        debug=False,
        enable_asserts=False,
        num_devices=NCORES,
    )

    # ---- External I/O ----
    xq = nc.dram_tensor("xq", [DM, L], BF16, kind="ExternalInput")  # q[b].T
    xk = nc.dram_tensor("xk", [DM, L], BF16, kind="ExternalInput")
    xv = nc.dram_tensor("xv", [DM, L], BF16, kind="ExternalInput")
    wq = nc.dram_tensor("wq", [DM, 2 * P], BF16, kind="ExternalInput")  # group cols
    wk = nc.dram_tensor("wk", [DM, 2 * P], BF16, kind="ExternalInput")
    wv = nc.dram_tensor("wv", [DM, 2 * P], BF16, kind="ExternalInput")
    wo = nc.dram_tensor("wo", [2, P, DM], BF16, kind="ExternalInput")  # [hp, 2h*64, dm]
    eT = nc.dram_tensor("eT", [2, P, L], BF16, kind="ExternalInput")   # [hp, 2h*64, r]
    bqk = nc.dram_tensor("bqk", [P, 4], F32, kind="ExternalInput")  # cols 0:2 bq, 2:4 bk
    bv_t = nc.dram_tensor("bv", [P, 2 * P], F32, kind="ExternalInput")  # row-replicated
    bo_t = nc.dram_tensor("bo", [P, DM], F32, kind="ExternalInput")     # row-replicated
    out = nc.dram_tensor("out", [L, DM], F32, kind="ExternalOutput")

    with tile.TileContext(nc) as tc:
        with (
            tc.tile_pool(name="persist", bufs=1) as pp,
            tc.tile_pool(name="work", bufs=2) as wp,
            tc.tile_pool(name="small", bufs=4) as sp,
        ):
            # ---- persistent SBUF tensors ----
            ident = pp.tile([P, P], BF16)
            make_identity(nc, ident)

            wq_sb = pp.tile([P, KT, 2 * P], BF16)
            nc.sync.dma_start(wq_sb, wq.ap().rearrange("(t p) c -> p t c", p=P))
            wk_sb = pp.tile([P, KT, 2 * P], BF16)
            nc.sync.dma_start(wk_sb, wk.ap().rearrange("(t p) c -> p t c", p=P))
            wv_sb = pp.tile([P, KT, 2 * P], BF16)
            nc.sync.dma_start(wv_sb, wv.ap().rearrange("(t p) c -> p t c", p=P))
            wo_sb = pp.tile([P, 2, DM], BF16)
            nc.sync.dma_start(wo_sb, wo.ap().rearrange("h p m -> p h m"))
            eT_sb = pp.tile([P, 2, L], BF16)
            nc.sync.dma_start(eT_sb, eT.ap().rearrange("h p r -> p h r"))
            bqk_sb = pp.tile([P, 4], F32)
            nc.sync.dma_start(bqk_sb, bqk.ap())
            bv_sb = pp.tile([P, 2 * P], F32)
            nc.sync.dma_start(bv_sb, bv_t.ap())
            bo_sb = pp.tile([P, DM], F32)
            nc.sync.dma_start(bo_sb, bo_t.ap())

            qhT = pp.tile([P, 2, L], BF16)   # [64*hl+d, hp, i]
            khT = pp.tile([P, 2, L], BF16)
            vh = pp.tile([P, NIT, HG, 66], BF16)  # [j in tile, jt, local head, d|1|pad]
            outT = pp.tile([P, 2, L], BF16)  # [64*hl+d, hp, i]

            # all-ones stationary for the softmax-denominator matmul:
            # lhsT [128, 64] of ones -> psum rows all equal to the denom
            ones64 = pp.tile([P, 64], BF16)
            nc.gpsimd.memset(ones64, 1.0)

            # ---- Stage 1: projections ----
            with (
                tc.tile_pool(name="xin", bufs=2) as xp,
                tc.tile_pool(name="ps1", bufs=4, space="PSUM") as ps1,
            ):
                for src, wsb, dst, bcol in ((xq, wq_sb, qhT, 0), (xk, wk_sb, khT, 2)):
                    xt = xp.tile([P, KT, L], BF16, tag="xin")
                    nc.sync.dma_start(xt, src.ap().rearrange("(t p) i -> p t i", p=P))
                    for hp in range(2):
                        for ic in range(L // 512):
                            ps = ps1.tile([P, 512], F32, tag="ps1")
                            for kt in range(KT):
                                nc.tensor.matmul(
                                    ps,
                                    wsb[:, kt, hp * P:(hp + 1) * P],
                                    xt[:, kt, ic * 512:(ic + 1) * 512],
                                    start=(kt == 0),
                                    stop=(kt == KT - 1),
                                )
                            nc.scalar.activation(
                                dst[:, hp, ic * 512:(ic + 1) * 512], ps,
                                mybir.ActivationFunctionType.Identity,
                                bias=bqk_sb[:, bcol + hp:bcol + hp + 1],
                            )
                # v projection: [keys on partitions, head-depth on free]
                xt = xp.tile([P, KT, L], BF16, tag="xin")
                nc.sync.dma_start(xt, xv.ap().rearrange("(t p) i -> p t i", p=P))
                for jt in range(NIT):
                    ps = ps1.tile([P, 2 * P], F32, tag="psv")
                    for kt in range(KT):
                        nc.tensor.matmul(
                            ps,
                            xt[:, kt, jt * P:(jt + 1) * P],
                            wv_sb[:, kt, :],
                            start=(kt == 0),
                            stop=(kt == KT - 1),
                        )
                    nc.vector.tensor_tensor(
                        vh[:, jt, :, 0:64],
                        ps.rearrange("p (l d) -> p l d", l=HG),
                        bv_sb.rearrange("p (l d) -> p l d", l=HG),
                        mybir.AluOpType.add,
                    )

            # ---- Stage 2: attention ----
            # The two heads of a pair are interleaved so adjacent K=64
            # matmuls target different PE row-groups and run concurrently.
            with (
                tc.tile_pool(name="psA", bufs=3, space="PSUM") as psAp,
                tc.tile_pool(name="psT", bufs=2, space="PSUM") as psTp,
                tc.tile_pool(name="psO", bufs=2, space="PSUM") as psOp,
                tc.tile_pool(name="psD", bufs=1, space="PSUM") as psDp,
            ):
                for it in range(NIT):
                    ncj = _ncj(it)
                    W = (it + 1) * P           # valid band width (r cols)
                    CW = ncj * 512 + 512       # buffer width incl. zero pad
                    i0 = it * P
                    r_lo = L - P - i0          # first relative index in band
                    for hp in range(2):
                        q_stat = [qhT[64 * hl:64 * hl + 64, hp, i0:i0 + P]
                                  for hl in (0, 1)]

                        # exp(Srel/8) bands, [query on partitions, r on free]
                        xse = [wp.tile([P, 2560], BF16, tag=f"xse{hl}", name=f"xse{hl}")
                               for hl in (0, 1)]
                        for cs in range(ncj):
                            n = min(512, W - cs * 512)
                            for hl in (0, 1):
                                pb = 64 * hl
                                ps = psAp.tile([P, 512], F32, tag="psA")
                                nc.tensor.matmul(
                                    ps[:, :n],
                                    q_stat[hl],
                                    eT_sb[pb:pb + 64, hp,
                                          r_lo + cs * 512:r_lo + cs * 512 + n],
                                    start=True, stop=True,
                                )
                                nc.scalar.activation(
                                    xse[hl][:, cs * 512:cs * 512 + n], ps[:, :n],
                                    mybir.ActivationFunctionType.Exp, scale=SCALE,
                                )
                        for hl in (0, 1):
                            nc.gpsimd.memset(xse[hl][:, W:CW], 0.0)

                        # exp(QK^T/8)
                        pqk = [wp.tile([P, 2048], BF16, tag=f"pqk{hl}", name=f"pqk{hl}")
                               for hl in (0, 1)]
                        for jc in range(ncj):
                            for hl in (0, 1):
                                pb = 64 * hl
                                ps = psAp.tile([P, 512], F32, tag="psA")
                                nc.tensor.matmul(
                                    ps,
                                    q_stat[hl],
                                    khT[pb:pb + 64, hp, jc * 512:(jc + 1) * 512],
                                    start=True, stop=True,
                                )
                                nc.scalar.activation(
                                    pqk[hl][:, jc * 512:(jc + 1) * 512], ps,
                                    mybir.ActivationFunctionType.Exp, scale=SCALE,
                                )

                        # skew (SBUF->SBUF diagonal DMA) and P = pqk * xsk
                        nj = ncj * 512
                        pm = []
                        for hl in (0, 1):
                            xsk = wp.tile([P, 2048], BF16, tag=f"xsk{hl}")
                            row_len = xse[hl].ap[0][0]
                            diag = bass.AP(
                                xse[hl].tensor, xse[hl].offset + 127,
                                [[row_len - 1, P], [1, nj]],
                            )
                            nc.sync.dma_start(xsk[:, :nj], diag)
                            pmt = wp.tile([P, 2048], BF16, tag=f"pm{hl}")
                            nc.vector.tensor_tensor(
                                pmt[:, :nj], pqk[hl][:, :nj], xsk[:, :nj],
                                mybir.AluOpType.mult,
                            )
                            pm.append(pmt)

                        # PV + denominator into column-half psum tiles:
                        # head hl occupies psum partitions [64*hl, 64*hl+64)
                        pso = psOp.tile([P, P], F32, tag="psO")
                        psd = psDp.tile([P, P], F32, tag="psD")
                        for jb in range(ncj):
                            for hl in (0, 1):
                                pb = 64 * hl
                                lh = 2 * hp + hl
                                pst = psTp.tile([P, 512], BF16, tag="psT")
                                for t in range(4):
                                    nc.tensor.transpose(
                                        pst[:, t * P:(t + 1) * P],
                                        pm[hl][:, jb * 512 + t * P:
                                               jb * 512 + (t + 1) * P],
                                        ident,
                                    )
                                pts = sp.tile([P, 512], BF16, tag="pts")
                                if (2 * jb + hl) % 2 == 0:
                                    nc.vector.tensor_copy(pts, pst)
                                else:
                                    nc.scalar.copy(pts, pst)
                                for t in range(4):
                                    jt = jb * 4 + t
                                    nc.tensor.matmul(
                                        pso[pb:pb + 64, :],
                                        vh[:, jt, lh, 0:64],
                                        pts[:, t * P:(t + 1) * P],
                                        start=(jt == 0),
                                        stop=(jt == ncj * 4 - 1),
                                        skip_group_check=True,
                                    )
                                    nc.tensor.matmul(
                                        psd[pb:pb + 64, :],
                                        ones64,
                                        pts[:, t * P:(t + 1) * P],
                                        start=(jt == 0),
                                        stop=(jt == ncj * 4 - 1),
                                        skip_group_check=True,
                                    )

                        # normalize both heads at once; writes land directly
                        # in the packed outT partition halves
                        rec_b = sp.tile([P, P], F32, tag="recb")
                        nc.vector.reciprocal_approx_fast(out=rec_b, in_=psd)
                        nc.vector.tensor_tensor(
                            outT[:, hp, i0:i0 + P], pso, rec_b,
                            mybir.AluOpType.mult,
                        )

            # ---- Stage 3: output projection (partial: this head group) ----
            with tc.tile_pool(name="ps3", bufs=2, space="PSUM") as ps3:
                for it in range(NIT):
                    for mc in range(DM // 512):
                        ps = ps3.tile([P, 512], F32, tag="ps3")
                        for hp in range(2):
                            nc.tensor.matmul(
                                ps,
                                outT[:, hp, it * P:(it + 1) * P],
                                wo_sb[:, hp, mc * 512:(mc + 1) * 512],
                                start=(hp == 0),
                                stop=(hp == 1),
                            )
                        osb = sp.tile([P, 512], F32, tag="osb")
                        nc.vector.tensor_tensor(
                            osb, ps, bo_sb[:, mc * 512:(mc + 1) * 512],
                            mybir.AluOpType.add,
                        )
                        nc.sync.dma_start(
                            out.ap()[it * P:(it + 1) * P, mc * 512:(mc + 1) * 512], osb
                        )
    nc.compile()
    return nc


def _prep_inputs(q, k, v, Wq, bq, Wk, bk, Wv, bv, Wo, bo, E):
    """Build the 8 per-core input maps (host-side shard + cast)."""
    in_maps = []
    for core in range(NCORES):
        b, g = core // HG, core % HG
        cols = slice(g * HG * D, (g + 1) * HG * D)  # this group's 256 cols
        # eT/wo packing: [hp, 64*hl + d, .]
        eTg = np.empty((2, P, L), NPBF16)
        wog = np.empty((2, P, DM), NPBF16)
        for hp in range(2):
            for hl in range(2):
                h = g * HG + 2 * hp + hl
                eTg[hp, 64 * hl:64 * hl + 64, :] = E[:, h * D:(h + 1) * D].T.astype(NPBF16)
                wog[hp, 64 * hl:64 * hl + 64, :] = Wo[h * D:(h + 1) * D, :].astype(NPBF16)
        bqk_a = np.empty((P, 4), np.float32)
        for hp in range(2):
            bqk_a[:, hp] = bq[g * HG * D + hp * P:g * HG * D + (hp + 1) * P]
            bqk_a[:, 2 + hp] = bk[g * HG * D + hp * P:g * HG * D + (hp + 1) * P]
        bo_full = bo if g == 0 else np.zeros_like(bo)
        in_maps.append({
            "xq": np.ascontiguousarray(q[b].T).astype(NPBF16),
            "xk": np.ascontiguousarray(k[b].T).astype(NPBF16),
            "xv": np.ascontiguousarray(v[b].T).astype(NPBF16),
            "wq": np.ascontiguousarray(Wq[:, cols]).astype(NPBF16),
            "wk": np.ascontiguousarray(Wk[:, cols]).astype(NPBF16),
            "wv": np.ascontiguousarray(Wv[:, cols]).astype(NPBF16),
            "wo": wog,
            "eT": eTg,
            "bqk": bqk_a,
            "bv": np.ascontiguousarray(
                np.broadcast_to(bv[None, cols], (P, 2 * P))).astype(np.float32),
            "bo": np.ascontiguousarray(
                np.broadcast_to(bo_full[None, :], (P, DM))).astype(np.float32),
        })
    return in_maps


def _reference_numpy(q, k, v, mask, Wq, bq, Wk, bk, Wv, bv, Wo, bo, E):
    """Exact fallback for non-causal masks (never hit in practice)."""
    def split_heads(x):
        return np.moveaxis(x.reshape(*x.shape[:-1], H, D), -2, -3)
    qh = split_heads(q @ Wq + bq)
    kh = split_heads(k @ Wk + bk)
    vv = split_heads(v @ Wv + bv)
    eh = split_heads(E)
    QKt = np.einsum("bhqd,bhkd->bhqk", qh, kh)
    X = np.einsum("bhqd,hkd->bhqk", qh, eh)
    pad = np.pad(X, [(0, 0)] * 3 + [(1, 0)])
    s = pad.reshape(B, H, -1)[:, :, L:].reshape(B, H, L, L)
    logits = (QKt + s) / np.sqrt(D) + mask * -1e9
    m = logits.max(-1, keepdims=True)
    p = np.exp(logits - m)
    p /= p.sum(-1, keepdims=True)
    o = np.einsum("bhqk,bhkd->bhqd", p, vv)
    o = np.moveaxis(o, -3, -2).reshape(B, L, DM)
    return (o @ Wo + bo).astype(np.float32)


def benchmark(inputs, iters=20):
    """Amortized wall-clock of the sharded NEFF execution (device-resident
    inputs, back-to-back async dispatch). Returns est. ns per execution."""
    global _PROG
    import time as _time
    import jax
    from jax.sharding import Mesh, PartitionSpec
    from jax.experimental.shard_map import shard_map
    import concourse.bass2jax as b2j
    import concourse.mybir as mb

    if _PROG is None:
        _PROG = build_program()
    nc = _PROG
    args = {n: np.asarray(inputs[n], np.float32)
            for n in ("q", "k", "v", "Wq", "bq", "Wk", "bk", "Wv", "bv",
                      "Wo", "bo", "E")}
    in_maps = _prep_inputs(**args)
    b2j.install_neuronx_cc_hook()

    partition_name = (nc.partition_id_tensor.name
                      if nc.partition_id_tensor else None)
    in_names, out_names, out_avals, zero_outs = [], [], [], []
    for alloc in nc.m.functions[0].allocations:
        if not isinstance(alloc, mb.MemoryLocationSet):
            continue
        name = alloc.memorylocations[0].name
        if alloc.kind == "ExternalInput":
            if name != partition_name:
                in_names.append(name)
        elif alloc.kind == "ExternalOutput":
            out_names.append(name)
            shape = tuple(alloc.tensor_shape)
            dtype = mb.dt.np(alloc.dtype)
            out_avals.append(jax.core.ShapedArray(shape, dtype))
            zero_outs.append(np.zeros(shape, dtype))
    n_params = len(in_names)
    n_outs = len(out_avals)
    all_names = in_names + out_names
    if partition_name is not None:
        all_names = all_names + [partition_name]

    def _body(*fargs):
        operands = list(fargs)
        if partition_name is not None:
            operands.append(b2j.partition_id_tensor())
        outs = b2j._bass_exec_p.bind(
            *operands, out_avals=tuple(out_avals), in_names=tuple(all_names),
            out_names=tuple(out_names), lowering_input_output_aliases=(),
            sim_require_finite=True, sim_require_nnan=True, nc=nc)
        return tuple(outs)

    devices = jax.devices()[:NCORES]
    mesh = Mesh(np.asarray(devices), ("core",))
    in_specs = (PartitionSpec("core"),) * (n_params + n_outs)
    out_specs = (PartitionSpec("core"),) * n_outs
    sharded = jax.jit(
        shard_map(_body, mesh=mesh, in_specs=in_specs, out_specs=out_specs,
                  check_rep=False),
        keep_unused=True)

    concat_in = [np.concatenate([np.asarray(in_maps[c][n])
                                 for c in range(NCORES)], axis=0)
                 for n in in_names]
    dev_in = [jax.device_put(a) for a in concat_in]
    concat_zero = [np.concatenate([z] * NCORES, axis=0) for z in zero_outs]

    dev_zero = [jax.device_put(z) for z in concat_zero]
    # warmup (compiles / caches)
    outs = sharded(*dev_in, *dev_zero)
    jax.block_until_ready(outs)

    t0 = _time.perf_counter()
    results = []
    for _ in range(iters):
        results.append(sharded(*dev_in, *dev_zero))
    jax.block_until_ready(results)
    t1 = _time.perf_counter()
    return (t1 - t0) / iters * 1e9


def kernel(**inputs):
    global _PROG, LAST_EXEC_NS
    args = {n: np.asarray(inputs[n], np.float32)
            for n in ("q", "k", "v", "Wq", "bq", "Wk", "bk", "Wv", "bv",
                      "Wo", "bo", "E")}
    mask = np.asarray(inputs["mask"], np.float32)

    causal = np.array_equal(mask, np.triu(np.ones((L, L), np.float32), k=1))
    if not causal:
        return _reference_numpy(mask=mask, **args)

    if _PROG is None:
        _PROG = build_program()
    in_maps = _prep_inputs(**args)
    trace = os.environ.get("KERNEL_TRACE", "0") == "1"
    try:
        res = run_bass_kernel_spmd(_PROG, in_maps, core_ids=list(range(NCORES)),
                                   trace=trace)
    except ModuleNotFoundError:
        # axon NTFF profiling hook unavailable in this container
        res = run_bass_kernel_spmd(_PROG, in_maps, core_ids=list(range(NCORES)),
                                   trace=False)
    LAST_EXEC_NS = res.exec_time_ns

    full = np.zeros((B, L, DM), np.float32)
    for core in range(NCORES):
        full[core // HG] += res.results[core]["out"]
    return full



# revision 2
# speedup vs baseline: 52.8944x; 52.8944x over previous
"""Trainium2 Bass kernel for nn_MultiHeadAttention_21251498181338.

Music-Transformer-style MHA with relative position embeddings (Huang et al.
skew trick), B=2, L=2048, D=1024, H=16, causal mask.

Sharding: 8 cores = 2 batches x 4-head groups (tensor parallel per head).
Each core computes q/k/v projections for its 4 heads, causal attention with
relative position logits, and a partial output projection (Wo row-split).
Partials are summed on the host during unshard.

Device-side structure (per core):
  - Projections produce qh^T/kh^T in [head-depth on partitions] layout and
    vh in [keys on partitions] layout, so no transposes are needed anywhere
    except for the attention probabilities themselves.
  - P = exp(QK^T/8) * exp(Srel/8): the additive logit split is computed
    multiplicatively so the relative-position term can be skew-aligned
    independently of QK^T.
  - The skew is a single SBUF->SBUF DMA per (head, q-tile) using a flat
    access pattern with partition step (row_len - 1): row i is read with a
    column offset of -i, which is exactly the Huang et al. pad/reshape
    trick. Columns beyond the valid relative-index range are zeroed, which
    also implements the causal mask for free (P = Pqk * 0 = 0 there).
  - PV uses TensorE transposes of P tiles; a parallel all-ones-stationary
    matmul produces the softmax denominators, replicated across psum
    partitions so the normalization is a single aligned vector multiply.
  - The two heads of each pair interleave their K=64 matmuls (different PE
    row-groups run concurrently) and share [128, P] psum tiles for PV and
    denominators via tile_position column halves, so both heads normalize
    in one op and land directly in the packed outT layout.
  - The attention output appears transposed [depth, queries], which is
    exactly the stationary-operand layout the output projection needs.
"""

import os
import sys

sys.path.insert(0, "/opt/trn_rl_repo")

import numpy as np
import ml_dtypes

import concourse.bass as bass
import concourse.mybir as mybir
import concourse.tile as tile
from concourse import bacc
from concourse.bass_utils import run_bass_kernel_spmd
from concourse.masks import make_identity

BF16 = mybir.dt.bfloat16
F32 = mybir.dt.float32
NPBF16 = ml_dtypes.bfloat16

B, L, DM, H, D = 2, 2048, 1024, 16, 64
HG = 4            # heads per core (head group)
NCORES = 8
P = 128
KT = DM // P      # 8 contraction tiles for projections
NIT = L // P      # 16 query tiles
SCALE = 1.0 / np.sqrt(D)  # 0.125

LAST_EXEC_NS = None

_PROG = None


def _ncj(it):
    # number of 512-wide key chunks for query tile `it` (causal)
    return it // 4 + 1


def build_program():
    nc = bacc.Bacc(
        "TRN2",
        target_bir_lowering=False,
        debug=False,
        enable_asserts=False,
        num_devices=NCORES,
    )

    # ---- External I/O ----
    xq = nc.dram_tensor("xq", [DM, L], BF16, kind="ExternalInput")  # q[b].T
    xk = nc.dram_tensor("xk", [DM, L], BF16, kind="ExternalInput")
    xv = nc.dram_tensor("xv", [DM, L], BF16, kind="ExternalInput")
    wq = nc.dram_tensor("wq", [DM, 2 * P], BF16, kind="ExternalInput")  # group cols
    wk = nc.dram_tensor("wk", [DM, 2 * P], BF16, kind="ExternalInput")
    wv = nc.dram_tensor("wv", [DM, 2 * P], BF16, kind="ExternalInput")
    wo = nc.dram_tensor("wo", [2, P, DM], BF16, kind="ExternalInput")  # [hp, 2h*64, dm]
    eT = nc.dram_tensor("eT", [2, P, L], BF16, kind="ExternalInput")   # [hp, 2h*64, r]
    bqk = nc.dram_tensor("bqk", [P, 4], F32, kind="ExternalInput")  # cols 0:2 bq, 2:4 bk
    bv_t = nc.dram_tensor("bv", [P, 2 * P], F32, kind="ExternalInput")  # row-replicated
    bo_t = nc.dram_tensor("bo", [P, DM], F32, kind="ExternalInput")     # row-replicated
    out = nc.dram_tensor("out", [L, DM], F32, kind="ExternalOutput")

    with tile.TileContext(nc) as tc:
        with (
            tc.tile_pool(name="persist", bufs=1) as pp,
            tc.tile_pool(name="work", bufs=2) as wp,
            tc.tile_pool(name="small", bufs=4) as sp,
        ):
            # ---- persistent SBUF tensors ----
            ident = pp.tile([P, P], BF16)
            make_identity(nc, ident)

            wq_sb = pp.tile([P, KT, 2 * P], BF16)
            nc.sync.dma_start(wq_sb, wq.ap().rearrange("(t p) c -> p t c", p=P))
            wk_sb = pp.tile([P, KT, 2 * P], BF16)
            nc.sync.dma_start(wk_sb, wk.ap().rearrange("(t p) c -> p t c", p=P))
            wv_sb = pp.tile([P, KT, 2 * P], BF16)
            nc.sync.dma_start(wv_sb, wv.ap().rearrange("(t p) c -> p t c", p=P))
            wo_sb = pp.tile([P, 2, DM], BF16)
            nc.sync.dma_start(wo_sb, wo.ap().rearrange("h p m -> p h m"))
            eT_sb = pp.tile([P, 2, L], BF16)
            nc.sync.dma_start(eT_sb, eT.ap().rearrange("h p r -> p h r"))
            bqk_sb = pp.tile([P, 4], F32)
            nc.sync.dma_start(bqk_sb, bqk.ap())
            bv_sb = pp.tile([P, 2 * P], F32)
            nc.sync.dma_start(bv_sb, bv_t.ap())
            bo_sb = pp.tile([P, DM], F32)
            nc.sync.dma_start(bo_sb, bo_t.ap())

            qhT = pp.tile([P, 2, L], BF16)   # [64*hl+d, hp, i]
            khT = pp.tile([P, 2, L], BF16)
            vh = pp.tile([P, NIT, HG, 66], BF16)  # [j in tile, jt, local head, d|1|pad]
            outT = pp.tile([P, 2, L], BF16)  # [64*hl+d, hp, i]

            # all-ones stationary for the softmax-denominator matmul:
            # lhsT [128, 64] of ones -> psum rows all equal to the denom
            ones64 = pp.tile([P, 64], BF16)
            nc.gpsimd.memset(ones64, 1.0)

            # ---- Stage 1: projections ----
            with (
                tc.tile_pool(name="xin", bufs=2) as xp,
                tc.tile_pool(name="ps1", bufs=4, space="PSUM") as ps1,
            ):
                for src, wsb, dst, bcol in ((xq, wq_sb, qhT, 0), (xk, wk_sb, khT, 2)):
                    xt = xp.tile([P, KT, L], BF16, tag="xin")
                    nc.sync.dma_start(xt, src.ap().rearrange("(t p) i -> p t i", p=P))
                    for hp in range(2):
                        for ic in range(L // 512):
                            ps = ps1.tile([P, 512], F32, tag="ps1")
                            for kt in range(KT):
                                nc.tensor.matmul(
                                    ps,
                                    wsb[:, kt, hp * P:(hp + 1) * P],
                                    xt[:, kt, ic * 512:(ic + 1) * 512],
                                    start=(kt == 0),
                                    stop=(kt == KT - 1),
                                )
                            nc.scalar.activation(
                                dst[:, hp, ic * 512:(ic + 1) * 512], ps,
                                mybir.ActivationFunctionType.Identity,
                                bias=bqk_sb[:, bcol + hp:bcol + hp + 1],
                            )
                # v projection: [keys on partitions, head-depth on free]
                xt = xp.tile([P, KT, L], BF16, tag="xin")
                nc.sync.dma_start(xt, xv.ap().rearrange("(t p) i -> p t i", p=P))
                for jt in range(NIT):
                    ps = ps1.tile([P, 2 * P], F32, tag="psv")
                    for kt in range(KT):
                        nc.tensor.matmul(
                            ps,
                            xt[:, kt, jt * P:(jt + 1) * P],
                            wv_sb[:, kt, :],
                            start=(kt == 0),
                            stop=(kt == KT - 1),
                        )
                    nc.vector.tensor_tensor(
                        vh[:, jt, :, 0:64],
                        ps.rearrange("p (l d) -> p l d", l=HG),
                        bv_sb.rearrange("p (l d) -> p l d", l=HG),
                        mybir.AluOpType.add,
                    )

            # ---- Stage 2: attention ----
            # The two heads of a pair are interleaved so adjacent K=64
            # matmuls target different PE row-groups and run concurrently.
            with (
                tc.tile_pool(name="psA", bufs=3, space="PSUM") as psAp,
                tc.tile_pool(name="psT", bufs=2, space="PSUM") as psTp,
                tc.tile_pool(name="psO", bufs=2, space="PSUM") as psOp,
                tc.tile_pool(name="psD", bufs=1, space="PSUM") as psDp,
            ):
                for it in range(NIT):
                    ncj = _ncj(it)
                    W = (it + 1) * P           # valid band width (r cols)
                    CW = ncj * 512 + 512       # buffer width incl. zero pad
                    i0 = it * P
                    r_lo = L - P - i0          # first relative index in band
                    for hp in range(2):
                        q_stat = [qhT[64 * hl:64 * hl + 64, hp, i0:i0 + P]
                                  for hl in (0, 1)]

                        # exp(Srel/8) bands, [query on partitions, r on free]
                        xse = [wp.tile([P, 2560], BF16, tag=f"xse{hl}", name=f"xse{hl}")
                               for hl in (0, 1)]
                        for cs in range(ncj):
                            n = min(512, W - cs * 512)
                            for hl in (0, 1):
                                pb = 64 * hl
                                ps = psAp.tile([P, 512], F32, tag="psA")
                                nc.tensor.matmul(
                                    ps[:, :n],
                                    q_stat[hl],
                                    eT_sb[pb:pb + 64, hp,
                                          r_lo + cs * 512:r_lo + cs * 512 + n],
                                    start=True, stop=True,
                                )
                                nc.scalar.activation(
                                    xse[hl][:, cs * 512:cs * 512 + n], ps[:, :n],
                                    mybir.ActivationFunctionType.Exp, scale=SCALE,
                                )
                        for hl in (0, 1):
                            nc.gpsimd.memset(xse[hl][:, W:CW], 0.0)

                        # exp(QK^T/8)
                        pqk = [wp.tile([P, 2048], BF16, tag=f"pqk{hl}", name=f"pqk{hl}")
                               for hl in (0, 1)]
                        for jc in range(ncj):
                            for hl in (0, 1):
                                pb = 64 * hl
                                ps = psAp.tile([P, 512], F32, tag="psA")
                                nc.tensor.matmul(
                                    ps,
                                    q_stat[hl],
                                    khT[pb:pb + 64, hp, jc * 512:(jc + 1) * 512],
                                    start=True, stop=True,
                                )
                                nc.scalar.activation(
                                    pqk[hl][:, jc * 512:(jc + 1) * 512], ps,
                                    mybir.ActivationFunctionType.Exp, scale=SCALE,
                                )

                        # skew (SBUF->SBUF diagonal DMA) and P = pqk * xsk
                        nj = ncj * 512
                        pm = []
                        for hl in (0, 1):
                            xsk = wp.tile([P, 2048], BF16, tag=f"xsk{hl}")
                            row_len = xse[hl].ap[0][0]
                            diag = bass.AP(
                                xse[hl].tensor, xse[hl].offset + 127,
                                [[row_len - 1, P], [1, nj]],
                            )
                            nc.sync.dma_start(xsk[:, :nj], diag)
                            pmt = wp.tile([P, 2048], BF16, tag=f"pm{hl}")
                            nc.vector.tensor_tensor(
                                pmt[:, :nj], pqk[hl][:, :nj], xsk[:, :nj],
                                mybir.AluOpType.mult,
                            )
                            pm.append(pmt)

                        # PV + denominator into column-half psum tiles:
                        # head hl occupies psum partitions [64*hl, 64*hl+64)
                        pso = psOp.tile([P, P], F32, tag="psO")
                        psd = psDp.tile([P, P], F32, tag="psD")
                        for jb in range(ncj):
                            for hl in (0, 1):
                                pb = 64 * hl
                                lh = 2 * hp + hl
                                pst = psTp.tile([P, 512], BF16, tag="psT")
                                for t in range(4):
                                    nc.tensor.transpose(
                                        pst[:, t * P:(t + 1) * P],
                                        pm[hl][:, jb * 512 + t * P:
                                               jb * 512 + (t + 1) * P],
                                        ident,
                                    )
                                pts = sp.tile([P, 512], BF16, tag="pts")
                                if (2 * jb + hl) % 2 == 0:
                                    nc.vector.tensor_copy(pts, pst)
                                else:
                                    nc.scalar.copy(pts, pst)
                                for t in range(4):
                                    jt = jb * 4 + t
                                    nc.tensor.matmul(
                                        pso[pb:pb + 64, :],
                                        vh[:, jt, lh, 0:64],
                                        pts[:, t * P:(t + 1) * P],
                                        start=(jt == 0),
                                        stop=(jt == ncj * 4 - 1),
                                        skip_group_check=True,
                                    )
                                    nc.tensor.matmul(
                                        psd[pb:pb + 64, :],
                                        ones64,
                                        pts[:, t * P:(t + 1) * P],
                                        start=(jt == 0),
                                        stop=(jt == ncj * 4 - 1),
                                        skip_group_check=True,
                                    )

                        # normalize both heads at once; writes land directly
                        # in the packed outT partition halves
                        rec_b = sp.tile([P, P], F32, tag="recb")
                        nc.vector.reciprocal_approx_fast(out=rec_b, in_=psd)
                        nc.vector.tensor_tensor(
                            outT[:, hp, i0:i0 + P], pso, rec_b,
                            mybir.AluOpType.mult,
                        )

            # ---- Stage 3: output projection (partial: this head group) ----
            with tc.tile_pool(name="ps3", bufs=2, space="PSUM") as ps3:
                for it in range(NIT):
                    for mc in range(DM // 512):
                        ps = ps3.tile([P, 512], F32, tag="ps3")
                        for hp in range(2):
                            nc.tensor.matmul(
                                ps,
                                outT[:, hp, it * P:(it + 1) * P],
                                wo_sb[:, hp, mc * 512:(mc + 1) * 512],
                                start=(hp == 0),
                                stop=(hp == 1),
                            )
                        osb = sp.tile([P, 512], F32, tag="osb")
                        nc.vector.tensor_tensor(
                            osb, ps, bo_sb[:, mc * 512:(mc + 1) * 512],
                            mybir.AluOpType.add,
                        )
                        nc.sync.dma_start(
                            out.ap()[it * P:(it + 1) * P, mc * 512:(mc + 1) * 512], osb
                        )
    nc.compile()
    return nc


def _prep_inputs(q, k, v, Wq, bq, Wk, bk, Wv, bv, Wo, bo, E):
    """Build the 8 per-core input maps (host-side shard + cast)."""
    in_maps = []
    for core in range(NCORES):
        b, g = core // HG, core % HG
        cols = slice(g * HG * D, (g + 1) * HG * D)  # this group's 256 cols
        # eT/wo packing: [hp, 64*hl + d, .]
        eTg = np.empty((2, P, L), NPBF16)
        wog = np.empty((2, P, DM), NPBF16)
        for hp in range(2):
            for hl in range(2):
                h = g * HG + 2 * hp + hl
                eTg[hp, 64 * hl:64 * hl + 64, :] = E[:, h * D:(h + 1) * D].T.astype(NPBF16)
                wog[hp, 64 * hl:64 * hl + 64, :] = Wo[h * D:(h + 1) * D, :].astype(NPBF16)
        bqk_a = np.empty((P, 4), np.float32)
        for hp in range(2):
            bqk_a[:, hp] = bq[g * HG * D + hp * P:g * HG * D + (hp + 1) * P]
            bqk_a[:, 2 + hp] = bk[g * HG * D + hp * P:g * HG * D + (hp + 1) * P]
        bo_full = bo if g == 0 else np.zeros_like(bo)
        in_maps.append({
            "xq": np.ascontiguousarray(q[b].T).astype(NPBF16),
            "xk": np.ascontiguousarray(k[b].T).astype(NPBF16),
            "xv": np.ascontiguousarray(v[b].T).astype(NPBF16),
            "wq": np.ascontiguousarray(Wq[:, cols]).astype(NPBF16),
            "wk": np.ascontiguousarray(Wk[:, cols]).astype(NPBF16),
            "wv": np.ascontiguousarray(Wv[:, cols]).astype(NPBF16),
            "wo": wog,
            "eT": eTg,
            "bqk": bqk_a,
            "bv": np.ascontiguousarray(
                np.broadcast_to(bv[None, cols], (P, 2 * P))).astype(np.float32),
            "bo": np.ascontiguousarray(
                np.broadcast_to(bo_full[None, :], (P, DM))).astype(np.float32),
        })
    return in_maps


def _reference_numpy(q, k, v, mask, Wq, bq, Wk, bk, Wv, bv, Wo, bo, E):
    """Exact fallback for non-causal masks (never hit in practice)."""
    def split_heads(x):
        return np.moveaxis(x.reshape(*x.shape[:-1], H, D), -2, -3)
    qh = split_heads(q @ Wq + bq)
    kh = split_heads(k @ Wk + bk)
    vv = split_heads(v @ Wv + bv)
    eh = split_heads(E)
    QKt = np.einsum("bhqd,bhkd->bhqk", qh, kh)
    X = np.einsum("bhqd,hkd->bhqk", qh, eh)
    pad = np.pad(X, [(0, 0)] * 3 + [(1, 0)])
    s = pad.reshape(B, H, -1)[:, :, L:].reshape(B, H, L, L)
    logits = (QKt + s) / np.sqrt(D) + mask * -1e9
    m = logits.max(-1, keepdims=True)
    p = np.exp(logits - m)
    p /= p.sum(-1, keepdims=True)
    o = np.einsum("bhqk,bhkd->bhqd", p, vv)
    o = np.moveaxis(o, -3, -2).reshape(B, L, DM)
    return (o @ Wo + bo).astype(np.float32)


def benchmark(inputs, iters=20):
    """Amortized wall-clock of the sharded NEFF execution (device-resident
    inputs, back-to-back async dispatch). Returns est. ns per execution."""
    global _PROG
    import time as _time
    import jax
    from jax.sharding import Mesh, PartitionSpec
    from jax.experimental.shard_map import shard_map
    import concourse.bass2jax as b2j
    import concourse.mybir as mb

    if _PROG is None:
        _PROG = build_program()
    nc = _PROG
    args = {n: np.asarray(inputs[n], np.float32)
            for n in ("q", "k", "v", "Wq", "bq", "Wk", "bk", "Wv", "bv",
                      "Wo", "bo", "E")}
    in_maps = _prep_inputs(**args)
    b2j.install_neuronx_cc_hook()

    partition_name = (nc.partition_id_tensor.name
                      if nc.partition_id_tensor else None)
    in_names, out_names, out_avals, zero_outs = [], [], [], []
    for alloc in nc.m.functions[0].allocations:
        if not isinstance(alloc, mb.MemoryLocationSet):
            continue
        name = alloc.memorylocations[0].name
        if alloc.kind == "ExternalInput":
            if name != partition_name:
                in_names.append(name)
        elif alloc.kind == "ExternalOutput":
            out_names.append(name)
            shape = tuple(alloc.tensor_shape)
            dtype = mb.dt.np(alloc.dtype)
            out_avals.append(jax.core.ShapedArray(shape, dtype))
            zero_outs.append(np.zeros(shape, dtype))
    n_params = len(in_names)
    n_outs = len(out_avals)
    all_names = in_names + out_names
    if partition_name is not None:
        all_names = all_names + [partition_name]

    def _body(*fargs):
        operands = list(fargs)
        if partition_name is not None:
            operands.append(b2j.partition_id_tensor())
        outs = b2j._bass_exec_p.bind(
            *operands, out_avals=tuple(out_avals), in_names=tuple(all_names),
            out_names=tuple(out_names), lowering_input_output_aliases=(),
            sim_require_finite=True, sim_require_nnan=True, nc=nc)
        return tuple(outs)

    devices = jax.devices()[:NCORES]
    mesh = Mesh(np.asarray(devices), ("core",))
    in_specs = (PartitionSpec("core"),) * (n_params + n_outs)
    out_specs = (PartitionSpec("core"),) * n_outs
    sharded = jax.jit(
        shard_map(_body, mesh=mesh, in_specs=in_specs, out_specs=out_specs,
                  check_rep=False),
        keep_unused=True)

    concat_in = [np.concatenate([np.asarray(in_maps[c][n])
                                 for c in range(NCORES)], axis=0)
                 for n in in_names]
    dev_in = [jax.device_put(a) for a in concat_in]
    concat_zero = [np.concatenate([z] * NCORES, axis=0) for z in zero_outs]

    dev_zero = [jax.device_put(z) for z in concat_zero]
    # warmup (compiles / caches)
    outs = sharded(*dev_in, *dev_zero)
    jax.block_until_ready(outs)

    t0 = _time.perf_counter()
    results = []
    for _ in range(iters):
        results.append(sharded(*dev_in, *dev_zero))
    jax.block_until_ready(results)
    t1 = _time.perf_counter()
    return (t1 - t0) / iters * 1e9


def kernel(**inputs):
    global _PROG, LAST_EXEC_NS
    args = {n: np.asarray(inputs[n], np.float32)
            for n in ("q", "k", "v", "Wq", "bq", "Wk", "bk", "Wv", "bv",
                      "Wo", "bo", "E")}
    mask = np.asarray(inputs["mask"], np.float32)

    causal = np.array_equal(mask, np.triu(np.ones((L, L), np.float32), k=1))
    if not causal:
        return _reference_numpy(mask=mask, **args)

    if _PROG is None:
        _PROG = build_program()
    in_maps = _prep_inputs(**args)
    trace = os.environ.get("KERNEL_TRACE", "0") == "1"
    try:
        res = run_bass_kernel_spmd(_PROG, in_maps, core_ids=list(range(NCORES)),
                                   trace=trace)
    except ModuleNotFoundError:
        # axon NTFF profiling hook unavailable in this container
        res = run_bass_kernel_spmd(_PROG, in_maps, core_ids=list(range(NCORES)),
                                   trace=False)
    LAST_EXEC_NS = res.exec_time_ns
    globals()["LAST_RESULTS"] = res

    full = np.zeros((B, L, DM), np.float32)
    for core in range(NCORES):
        full[core // HG] += res.results[core]["out"]
    return full

